# revision 12
# baseline (speedup 1.0000x reference)
"""Cross-attention kernel for Trainium2, data-parallel over batch on 8 NeuronCores.

Reference computation (per batch element b):
    lat = LN(latent_q[b]) ; inp = LN(input_kv[b])
    Q = lat @ W_Q ; K = inp @ W_K ; V = inp @ W_V      (8 heads x 128 dims)
    out[b] = softmax(Q K^T / sqrt(128)) V @ W_O

Sharding: batch B=8 -> one batch element per core, zero collectives.

Device program layout choices (per core):
  - All matmul operands bf16 (PE: 1 cycle/row), accumulation fp32 in PSUM.
  - LN gamma folded into W_Q/W_K/W_V on host; LN beta folded into per-output
    -channel bias vectors (t_q/t_k/t_v) applied during PSUM->SBUF copies.
  - x_kv is streamed in 32 chunks of 512 rows: LN (bn_stats) -> PE transpose
    to channel-major -> K^T/V projections -> S^T = K Q^T per head -> exp on
    ScalarE -> O'^T accumulated over chunks; softmax denominator accumulated
    as elementwise sums of exp tiles (partition-reduced once at the end with
    a ones-matmul).
  - Normalization by 1/l happens once at the end, before the W_O projection.
"""

import numpy as np
import ml_dtypes

import concourse.bass as bass
import concourse.mybir as mybir
import concourse.tile as tile
from concourse import bacc
from concourse.bass_utils import run_bass_kernel_spmd
from concourse.masks import make_identity

AF = mybir.ActivationFunctionType
DT = mybir.dt

B = 8
LQ = 512
LKV = 16384
DLAT = 1024
DIN = 768
QK_CH = 1024
V_CH = 1024
OUT_CH = 1024
H = 8
DH = 128
P = 128
EPS = 1e-5
SCALE = float(1.0 / np.sqrt(DH))

CHUNK = 512               # kv rows per chunk
N_LQ_T = LQ // P          # 4 q sub-tiles
N_LAT_S = DLAT // P       # 8 latent channel sub-tiles
N_IN_S = DIN // P         # 6 input channel sub-tiles
N_VC_S = V_CH // P        # 8


def _ln_apply(nc, stats_pool, x_ap, width, n_half, out_ap, eps_ap=None):
    """Emit LN (zero mean / unit variance) for one [128, width] fp32 tile,
    writing the normalized bf16 values into out_ap. gamma/beta are folded
    into the projection weights by the host. width//n_half must be <= 512."""
    half = width // n_half
    st = stats_pool.tile([P, 6 * n_half], DT.float32, tag="bnst")
    for i in range(n_half):
        nc.vector.bn_stats(st[:, 6 * i : 6 * i + 6], x_ap[:, i * half : (i + 1) * half])
    mv = stats_pool.tile([P, 2], DT.float32, tag="bnmv")
    nc.vector.bn_aggr(mv[:], st[:])
    # rsqrt(var+eps) as exp(-0.5*ln(var+eps)): keeps ScalarE on the single
    # natural_log_exp_and_others table set (no ~2.7us table swaps per chunk).
    lnv = stats_pool.tile([P, 1], DT.float32, tag="bnln")
    nc.scalar.activation(lnv[:], mv[:, 1:2], AF.Ln, bias=eps_ap)
    inv = stats_pool.tile([P, 1], DT.float32, tag="bninv")
    nc.scalar.activation(inv[:], lnv[:], AF.Exp, scale=-0.5)
    nmi = stats_pool.tile([P, 1], DT.float32, tag="bnnmi")
    nc.vector.tensor_mul(nmi[:], mv[:, 0:1], inv[:])
    nc.vector.tensor_scalar_mul(nmi[:], nmi[:], -1.0)
    nc.scalar.activation(out_ap, x_ap, AF.Identity, bias=nmi[:], scale=inv[:])


def build_program(lkv=LKV, reps=1):
    """Build the per-core Bass program. reps>1 wraps the body in a HW loop
    (each iteration recomputes the full output; used for wall-clock timing)."""
    n_chunks = lkv // CHUNK
    n_kv_t = CHUNK // P   # 4

    nc = bacc.Bacc()
    lq_d = nc.dram_tensor("lq", [LQ, DLAT], DT.float32, kind="ExternalInput")
    xkv_d = nc.dram_tensor("xkv", [lkv, DIN], DT.float32, kind="ExternalInput")
    wq_d = nc.dram_tensor("wq", [DLAT, QK_CH], DT.bfloat16, kind="ExternalInput")
    wk_d = nc.dram_tensor("wk", [DIN, QK_CH], DT.bfloat16, kind="ExternalInput")
    wv_d = nc.dram_tensor("wv", [DIN, V_CH], DT.bfloat16, kind="ExternalInput")
    wo_d = nc.dram_tensor("wo", [V_CH, OUT_CH], DT.bfloat16, kind="ExternalInput")
    tq_d = nc.dram_tensor("tq", [P, H], DT.float32, kind="ExternalInput")
    tk_d = nc.dram_tensor("tk", [P, H], DT.float32, kind="ExternalInput")
    tvb_d = nc.dram_tensor("tvb", [P, V_CH], DT.bfloat16, kind="ExternalInput")
    out_d = nc.dram_tensor("out", [LQ, OUT_CH], DT.float32, kind="ExternalOutput")

    with tile.TileContext(nc) as tc:
        with (
            tc.tile_pool(name="weights", bufs=1) as wpool,
            tc.tile_pool(name="persist", bufs=1) as perpool,
            tc.tile_pool(name="xin", bufs=2) as xpool,
            tc.tile_pool(name="xn", bufs=2) as xnpool,
            tc.tile_pool(name="xnt", bufs=2) as xntpool,
            tc.tile_pool(name="kt", bufs=2) as ktpool,
            tc.tile_pool(name="vt", bufs=2) as vpool,
            tc.tile_pool(name="pt", bufs=6) as ptpool,
            tc.tile_pool(name="stats", bufs=3) as stats_pool,
            tc.tile_pool(name="small", bufs=2) as smpool,
            tc.tile_pool(name="tpsum", bufs=2, space="PSUM") as tpsum,
            tc.tile_pool(name="kvpsum", bufs=2, space="PSUM") as kvpsum,
            tc.tile_pool(name="spsum", bufs=2, space="PSUM") as spsum,
            tc.tile_pool(name="opsum", bufs=2, space="PSUM") as opsum,
        ):
            # ---- weight/constant tiles (DMAs emitted inside body) ----
            wq_sb = wpool.tile([P, N_LAT_S, QK_CH], DT.bfloat16)
            wk_sb = wpool.tile([P, N_IN_S, QK_CH], DT.bfloat16)
            wv_sb = wpool.tile([P, N_IN_S, V_CH], DT.bfloat16)
            wo_sb = wpool.tile([P, N_VC_S, OUT_CH], DT.bfloat16)
            tq_sb = wpool.tile([P, H], DT.float32)
            tk_sb = wpool.tile([P, H], DT.float32)
            tvb_sb = wpool.tile([P, V_CH], DT.bfloat16)
            ident = wpool.tile([P, P], DT.bfloat16)
            ones_f32 = wpool.tile([P, 1], DT.float32)
            ones_row = wpool.tile([1, P], DT.float32)
            eps_sb = wpool.tile([P, 1], DT.float32)

            q_sb = perpool.tile([P, H, LQ], DT.bfloat16)
            o_acc = perpool.tile([P, H, LQ], DT.float32)
            l_acc = perpool.tile([P, H, LQ], DT.float32)

            def body():
                nc.sync.dma_start(wq_sb[:], wq_d[:].rearrange("(s p) n -> p s n", p=P))
                nc.sync.dma_start(wk_sb[:], wk_d[:].rearrange("(s p) n -> p s n", p=P))
                nc.sync.dma_start(wv_sb[:], wv_d[:].rearrange("(s p) n -> p s n", p=P))
                nc.sync.dma_start(wo_sb[:], wo_d[:].rearrange("(s p) n -> p s n", p=P))
                nc.sync.dma_start(tq_sb[:], tq_d[:])
                nc.sync.dma_start(tk_sb[:], tk_d[:])
                nc.sync.dma_start(tvb_sb[:], tvb_d[:])
                make_identity(nc, ident[:])
                nc.gpsimd.memset(ones_f32[:], 1.0)
                nc.gpsimd.memset(ones_row[:], 1.0)
                nc.gpsimd.memset(eps_sb[:], EPS)
                nc.gpsimd.memset(o_acc[:], 0.0)
                nc.gpsimd.memset(l_acc[:], 0.0)

                # ---------- prologue: latent LN -> transpose -> Q^T ----------
                latnT = ktpool.tile([P, N_LAT_S, LQ], DT.bfloat16, tag="kT")
                for t in range(N_LQ_T):
                    lat_t = xpool.tile([P, DLAT], DT.float32, tag="x")
                    nc.sync.dma_start(
                        lat_t[:],
                        lq_d[:].rearrange("(t p) n -> t p n", p=P)[t],
                    )
                    latn = xnpool.tile([P, DLAT], DT.bfloat16, tag="xn")
                    _ln_apply(nc, stats_pool, lat_t[:], DLAT, 2, latn[:], eps_sb[:])
                    for s in range(N_LAT_S):
                        ps = tpsum.tile([P, P], DT.bfloat16, tag="tp")
                        nc.tensor.transpose(
                            ps[:], latn[:, s * P : (s + 1) * P], ident[:]
                        )
                        nc.vector.tensor_copy(
                            latnT[:, s, t * P : (t + 1) * P], ps[:]
                        )
                for h in range(H):
                    qps = kvpsum.tile([P, LQ], DT.float32, tag="kv")
                    for s in range(N_LAT_S):
                        nc.tensor.matmul(
                            qps[:],
                            wq_sb[:, s, h * DH : (h + 1) * DH],
                            latnT[:, s, :],
                            start=(s == 0),
                            stop=(s == N_LAT_S - 1),
                        )
                    nc.scalar.activation(
                        q_sb[:, h, :], qps[:], AF.Identity, bias=tq_sb[:, h : h + 1]
                    )

                # ---------- main loop over kv chunks ----------
                xkv_r = xkv_d[:].rearrange("(c t p) ch -> c p t ch", t=n_kv_t, p=P)
                for c in range(n_chunks):
                    x_t = xpool.tile([P, n_kv_t, DIN], DT.float32, tag="x")
                    nc.sync.dma_start(x_t[:], xkv_r[c])
                    xn_t = xnpool.tile([P, n_kv_t, DIN], DT.bfloat16, tag="xn")
                    for t in range(n_kv_t):
                        _ln_apply(
                            nc, stats_pool, x_t[:, t, :], DIN, 2,
                            xn_t[:, t, :], eps_sb[:],
                        )
                    # transpose to channel-major [128, 6, CHUNK]
                    xnT = xntpool.tile([P, N_IN_S, CHUNK], DT.bfloat16)
                    for t in range(n_kv_t):
                        for s in range(N_IN_S):
                            ps = tpsum.tile([P, P], DT.bfloat16, tag="tp")
                            nc.tensor.transpose(
                                ps[:], xn_t[:, t, s * P : (s + 1) * P], ident[:]
                            )
                            nc.vector.tensor_copy(
                                xnT[:, s, t * P : (t + 1) * P], ps[:]
                            )
                    # K^T: [dh, CHUNK] per head
                    kT = ktpool.tile([P, H, CHUNK], DT.bfloat16, tag="kT")
                    for h in range(H):
                        kps = kvpsum.tile([P, CHUNK], DT.float32, tag="kv")
                        for s in range(N_IN_S):
                            nc.tensor.matmul(
                                kps[:],
                                wk_sb[:, s, h * DH : (h + 1) * DH],
                                xnT[:, s, :],
                                start=(s == 0),
                                stop=(s == N_IN_S - 1),
                            )
                        nc.scalar.activation(
                            kT[:, h, :], kps[:], AF.Identity, bias=tk_sb[:, h : h + 1]
                        )
                    # V natural: [CHUNK(4x128), V_CH]
                    v_t = vpool.tile([P, n_kv_t, V_CH], DT.bfloat16)
                    for t in range(n_kv_t):
                        for nf in range(V_CH // 512):
                            vps = kvpsum.tile([P, 512], DT.float32, tag="kv")
                            for s in range(N_IN_S):
                                nc.tensor.matmul(
                                    vps[:],
                                    xnT[:, s, t * P : (t + 1) * P],
                                    wv_sb[:, s, nf * 512 : (nf + 1) * 512],
                                    start=(s == 0),
                                    stop=(s == N_IN_S - 1),
                                )
                            nc.vector.tensor_add(
                                v_t[:, t, nf * 512 : (nf + 1) * 512],
                                vps[:],
                                tvb_sb[:, nf * 512 : (nf + 1) * 512],
                            )
                    # attention: S^T tiles -> exp -> L/O accumulation
                    for h in range(H):
                        ops = opsum.tile([P, LQ], DT.float32, tag="o")
                        for t in range(n_kv_t):
                            sps = spsum.tile([P, LQ], DT.float32, tag="s")
                            nc.tensor.matmul(
                                sps[:],
                                kT[:, h, t * P : (t + 1) * P],
                                q_sb[:, h, :],
                                start=True,
                                stop=True,
                            )
                            pT = ptpool.tile([P, LQ], DT.bfloat16)
                            nc.scalar.activation(pT[:], sps[:], AF.Exp, scale=SCALE)
                            nc.vector.tensor_add(l_acc[:, h, :], l_acc[:, h, :], pT[:])
                            nc.tensor.matmul(
                                ops[:],
                                v_t[:, t, h * DH : (h + 1) * DH],
                                pT[:],
                                start=(t == 0),
                                stop=(t == n_kv_t - 1),
                            )
                        nc.vector.tensor_add(o_acc[:, h, :], o_acc[:, h, :], ops[:])

                # ---------- epilogue: normalize, project with W_O ----------
                o_n = ktpool.tile([P, H, LQ], DT.bfloat16, tag="kT")
                for h in range(H):
                    lps = spsum.tile([P, LQ], DT.float32, tag="s")
                    nc.tensor.matmul(
                        lps[0:1, :], ones_f32[:], l_acc[:, h, :], start=True, stop=True
                    )
                    rl = smpool.tile([1, LQ], DT.float32, tag="rl")
                    nc.vector.reciprocal(rl[:], lps[0:1, :])
                    bps = opsum.tile([P, LQ], DT.float32, tag="o")
                    nc.tensor.matmul(
                        bps[:], ones_row[:], rl[:], start=True, stop=True
                    )
                    nc.vector.tensor_mul(o_n[:, h, :], o_acc[:, h, :], bps[:])
                out_sb = xpool.tile([P, N_LQ_T, OUT_CH], DT.float32, tag="x")
                for qt in range(N_LQ_T):
                    for nf in range(OUT_CH // 512):
                        octile = kvpsum.tile([P, 512], DT.float32, tag="kv")
                        for s in range(N_VC_S):
                            nc.tensor.matmul(
                                octile[:],
                                o_n[:, s, qt * P : (qt + 1) * P],
                                wo_sb[:, s, nf * 512 : (nf + 1) * 512],
                                start=(s == 0),
                                stop=(s == N_VC_S - 1),
                            )
                        nc.scalar.activation(
                            out_sb[:, qt, nf * 512 : (nf + 1) * 512], octile[:], AF.Copy
                        )
                nc.sync.dma_start(
                    out_d[:].rearrange("(t p) n -> p t n", p=P), out_sb[:]
                )

            if reps == 1:
                body()
            else:
                with tc.For_i(0, reps, 1) as _i:
                    body()

    nc.compile()
    return nc


def host_prep(W_Q, W_K, W_V, W_O, ln_lat_g, ln_lat_b, ln_in_g, ln_in_b):
    """Fold LN affine params into weights; returns device input dict pieces."""
    bf16 = ml_dtypes.bfloat16
    wq = (ln_lat_g[:, None].astype(np.float64) * W_Q.astype(np.float64)).astype(bf16)
    wk = (ln_in_g[:, None].astype(np.float64) * W_K.astype(np.float64)).astype(bf16)
    wv = (ln_in_g[:, None].astype(np.float64) * W_V.astype(np.float64)).astype(bf16)
    wo = W_O.astype(bf16)
    tq = (ln_lat_b.astype(np.float64) @ W_Q.astype(np.float64)).astype(np.float32)
    tk = (ln_in_b.astype(np.float64) @ W_K.astype(np.float64)).astype(np.float32)
    tv = (ln_in_b.astype(np.float64) @ W_V.astype(np.float64)).astype(np.float32)
    # t_q/t_k laid out [dh-partition, head]; t_v broadcast to all partitions
    tq_l = np.ascontiguousarray(tq.reshape(H, DH).T)
    tk_l = np.ascontiguousarray(tk.reshape(H, DH).T)
    tvb = np.ascontiguousarray(np.broadcast_to(tv.astype(bf16), (P, V_CH)))
    return dict(wq=wq, wk=wk, wv=wv, wo=wo, tq=tq_l, tk=tk_l, tvb=tvb)


_prog_cache = {}


def _get_program():
    key = "main"
    if key not in _prog_cache:
        _prog_cache[key] = build_program()
    return _prog_cache[key]


def kernel(latent_q, input_kv, W_Q, W_K, W_V, W_O,
           ln_lat_g, ln_lat_b, ln_in_g, ln_in_b):
    shared = host_prep(W_Q, W_K, W_V, W_O, ln_lat_g, ln_lat_b, ln_in_g, ln_in_b)
    nc = _get_program()
    in_maps = [
        dict(
            lq=np.ascontiguousarray(latent_q[b]),
            xkv=np.ascontiguousarray(input_kv[b]),
            **shared,
        )
        for b in range(B)
    ]
    res = run_bass_kernel_spmd(nc, in_maps, list(range(B)))
    out = np.stack([res.results[b]["out"] for b in range(B)])
    return out.astype(np.float32)
